# revision 28
# baseline (speedup 1.0000x reference)
"""Trainium2 Bass kernel for BiLSTM text classifier — 16-way time-split.

16 units = 2 directions x 8 time segments; each of the 8 cores interleaves
TWO units (one per direction) step by step, so the two dependency chains
hide each other's latency and the core becomes throughput-bound instead of
chain-bound.  Each unit runs a 156-step window (124-156 real + 32 warmup)
over ALL 128 examples.  A 32-step zero-state warmup reproduces the true
LSTM state to ~2e-7 (forget gates average ~sigmoid(+-1): state decays ~2x
per step).

BN1 is folded into the input weights on the host (exact batch stats via
bincount).  h is stored halved (h' = sig_o*(sig(2c)-0.5)) with the 2x
folded into U; BN2 consumes h' with eps/4.  Gates are host-permuted to
[cc,i,f,o] for a split sigmoid.  The embedding gather is pipelined into
the scan (indirect DMA), with the x-transpose on the PE.  Final states are
exchanged with one masked AllReduce; every core then computes BN2 + dense
+ softmax for the full batch and the host reads core 0's output.
"""

import os
import sys

sys.path.insert(0, "/opt/trn_rl_repo")

import ml_dtypes
import numpy as np

from concourse import bacc, bass, mybir, tile
from concourse.bass import IndirectOffsetOnAxis
from concourse.bass_utils import run_bass_kernel_spmd
from concourse.masks import make_identity

F32 = mybir.dt.float32
BF16 = mybir.dt.bfloat16
I32 = mybir.dt.int32
AF = mybir.ActivationFunctionType
OP = mybir.AluOpType
AX = mybir.AxisListType

# Problem dims
B, T, E, H, ODIM, VOCAB = 128, 1024, 128, 128, 10, 100000
G4 = 4 * H  # 512
NCORES = 8
UPC = 2                      # units per core (slot 0 = fwd, slot 1 = bwd)
NSEGD = NCORES               # 8 equal segments, one per core
SEG = T // NSEGD             # 128
WARM = 24
L = SEG + WARM               # 152 steps per unit
# x_T block layout per core: [0..SEG) shared real blocks (block b = step
# t=SEG*c+b), [SEG..SEG+WARM) fwd warmup, [SEG+WARM..SEG+2W) bwd warmup.
# Core c runs fwd AND bwd over ITS OWN segment, so the real blocks are
# gathered once and consumed by both units (bwd reads them in reverse).
# Edge warmups (fwd on core 0, bwd on core 7) use id=0 dummy blocks whose
# all-masked steps carry the exact zero initial state.
NBLK = SEG + 2 * WARM        # 176 gathered blocks per core


def blkmap(u, j):
    if u == 0:  # fwd
        return SEG + j if j < WARM else j - WARM
    return SEG + WARM + j if j < WARM else SEG - 1 - (j - WARM)
BN_EPS = 1e-3

TRACE = False
TRACE_DIR = None
LAST_RESULT = {}


def build_program(mask_sched):
    """mask_sched: sorted list of (slot, local step) pairs (union over
    cores) needing masked-carry fixups; per-core data arrives via 'mfix'."""
    nc = bacc.Bacc("TRN2", target_bir_lowering=False, debug=False,
                   num_devices=NCORES)

    NFIX = len(mask_sched)

    ids_d = nc.dram_tensor("ids", [128, NBLK], I32, kind="ExternalInput")
    emb_d = nc.dram_tensor("emb", [VOCAB, E], BF16, kind="ExternalInput")
    W_d = [nc.dram_tensor(f"W{u}", [E, G4], BF16, kind="ExternalInput")
           for u in range(UPC)]
    U_d = [nc.dram_tensor(f"U{u}", [H, G4], BF16, kind="ExternalInput")
           for u in range(UPC)]
    Bp_d = [nc.dram_tensor(f"Bp{u}", [4, 128], BF16, kind="ExternalInput")
            for u in range(UPC)]
    sel_d = nc.dram_tensor("sel", [H, 2 * UPC], F32, kind="ExternalInput")
    g2_d = nc.dram_tensor("g2", [H, 2], F32, kind="ExternalInput")
    be2_d = nc.dram_tensor("be2", [H, 2], F32, kind="ExternalInput")
    Wd0_d = nc.dram_tensor("Wd0", [H, ODIM], BF16, kind="ExternalInput")
    Wd1_d = nc.dram_tensor("Wd1", [H, ODIM], BF16, kind="ExternalInput")
    bd_d = nc.dram_tensor("bd", [B, ODIM], F32, kind="ExternalInput")
    if NFIX:
        mfix_d = nc.dram_tensor("mfix", [NFIX * 128, B], mybir.dt.uint8,
                                kind="ExternalInput")
    out_d = nc.dram_tensor("out", [B, ODIM], F32, kind="ExternalOutput")

    with tile.TileContext(nc) as tc:
        with (
            tc.tile_pool(name="const", bufs=1) as cp,
            tc.tile_pool(name="xt", bufs=1) as xp,
            tc.tile_pool(name="state", bufs=1) as sp,
            tc.tile_pool(name="step", bufs=2) as stp,
            tc.tile_pool(name="nat", bufs=64) as natp,
            tc.tile_pool(name="dram", bufs=1, space="DRAM") as dp,
        ):
            ids_sb = cp.tile([128, NBLK], I32)
            x_T = xp.tile([E, NBLK * 128], BF16)
            wq = [cp.tile([E, G4], BF16, tag=f"w{u}", name=f"w{u}")
                  for u in range(UPC)]
            uq = [cp.tile([H, G4], BF16, tag=f"u{u}", name=f"u{u}")
                  for u in range(UPC)]
            Bp = [cp.tile([4, 128], BF16, tag=f"Bp{u}", name=f"Bp{u}")
                  for u in range(UPC)]
            Gind = cp.tile([4, G4], BF16)
            sel_sb = cp.tile([H, 2 * UPC], F32)
            wdq = [cp.tile([H, ODIM], BF16, tag=f"wd{d}", name=f"wd{d}")
                   for d in range(2)]
            bd_sb = cp.tile([B, ODIM], F32)
            g2_sb = cp.tile([H, 2], F32)
            be2_sb = cp.tile([H, 2], F32)
            if NFIX:
                mfix_sb = cp.tile([128, NFIX * B], mybir.dt.uint8)

            h_u = [sp.tile([H, B], BF16, tag=f"h{u}", name=f"h{u}")
                   for u in range(UPC)]
            c_u = [sp.tile([H, B], F32, tag=f"c{u}", name=f"c{u}")
                   for u in range(UPC)]
            ident = cp.tile([128, 128], BF16)
            make_identity(nc, ident[:])

            nc.sync.dma_start(ids_sb[:], ids_d[:, :])
            for u in range(UPC):
                nc.sync.dma_start(wq[u][:], W_d[u][:, :])
                nc.sync.dma_start(uq[u][:], U_d[u][:, :])
                nc.sync.dma_start(Bp[u][:], Bp_d[u][:, :])
            nc.sync.dma_start(sel_sb[:], sel_d[:, :])
            nc.sync.dma_start(wdq[0][:], Wd0_d[:, :])
            nc.sync.dma_start(wdq[1][:], Wd1_d[:, :])
            nc.sync.dma_start(bd_sb[:], bd_d[:, :])
            nc.sync.dma_start(g2_sb[:], g2_d[:, :])
            nc.sync.dma_start(be2_sb[:], be2_d[:, :])
            if NFIX:
                for r in range(NFIX):
                    nc.sync.dma_start(
                        mfix_sb[:, r * B:(r + 1) * B],
                        mfix_d[r * 128:(r + 1) * 128, :])
            for u in range(UPC):
                nc.vector.memset(h_u[u][:], 0.0)
                nc.vector.memset(c_u[u][:], 0.0)

            # gate-block indicator for the rank-4 bias matmul
            nc.gpsimd.memset(Gind[:], 0.0)
            nc.gpsimd.affine_select(
                out=Gind[:].rearrange("p (q r) -> p q r", q=4),
                in_=Gind[:].rearrange("p (q r) -> p q r", q=4),
                compare_op=OP.not_equal,
                fill=1.0,
                base=0,
                pattern=[[1, 4], [0, 128]],
                channel_multiplier=-1,
            )

            # ---- gather (consumption order), pipelined into the scan ----
            gorder = []
            for i in range(WARM):                 # warmups first
                gorder += [SEG + i, SEG + WARM + i]
            for b in range(SEG // 2):             # shared from both ends
                gorder += [b, SEG - 1 - b]
            xnats = {}
            for blk in gorder:
                xnat = natp.tile([128, E], BF16, tag="xnat", name="xnat")
                nc.gpsimd.indirect_dma_start(
                    out=xnat[:],
                    out_offset=None,
                    in_=emb_d[:, :],
                    in_offset=IndirectOffsetOnAxis(
                        ap=ids_sb[:, blk:blk + 1], axis=0),
                )
                xnats[blk] = xnat

            fix_map = {}
            for r, key in enumerate(mask_sched):
                fix_map[tuple(key)] = r

            # ---- the interleaved scan ----
            with (
                tc.tile_pool(name="pstep", bufs=4, space="PSUM") as pstep,
                tc.tile_pool(name="pso", bufs=1, space="PSUM") as po,
                tc.tile_pool(name="ptr", bufs=2, space="PSUM") as ptrp,
                tc.tile_pool(name="pdum", bufs=1, space="PSUM") as pdum_p,
            ):
                # Write-only dummy matmuls keep the PE activity monitor
                # above its throttle threshold (HAM): without them the
                # array duty cycle is ~35% and the clock stays at 1.2 GHz.
                pdum = pdum_p.tile([128, G4], F32, space="PSUM",
                                   tag="pdum", name="pdum")
                def emit_tr(blk):
                    pt = ptrp.tile([128, 128], BF16, space="PSUM", tag="pt",
                                   name="pt")
                    nc.tensor.transpose(pt[:], xnats[blk][:], ident[:])
                    col = blk * 128
                    nc.vector.tensor_copy(x_T[:, col:col + 128], pt[:])

                def emit_wx(u, j):
                    ps = pstep.tile([128, G4], F32, space="PSUM", tag="ps",
                                    name="ps")
                    col = blkmap(u, j) * 128
                    toks = x_T[:, col:col + 128]
                    for g in range(4):
                        nc.tensor.matmul(
                            ps[:, g * 128:(g + 1) * 128],
                            wq[u][:, g * 128:(g + 1) * 128], toks,
                            start=(g == 0), stop=False,
                            skip_group_check=True)
                    nc.tensor.matmul(ps[:], Bp[u][:], Gind[:],
                                     start=False, stop=False,
                                     skip_group_check=True)
                    return ps

                def recurrent_step(u, j, ps):
                    for g in range(4):
                        nc.tensor.matmul(
                            ps[:, g * 128:(g + 1) * 128],
                            uq[u][:, g * 128:(g + 1) * 128], h_u[u][:],
                            start=False, stop=True,
                            skip_group_check=True)
                    nc.tensor.matmul(pdum[:], uq[u][:, 0:128], wq[u][:],
                                     start=True, stop=True,
                                     skip_group_check=True)
                    nc.tensor.matmul(pdum[:], wq[u][:, 0:128], uq[u][:],
                                     start=True, stop=True,
                                     skip_group_check=True)
                    s_t = stp.tile([128, G4], F32, tag=f"s{u}",
                                   name=f"s{u}")
                    nc.scalar.activation(s_t[:, 0:256], ps[:, 0:256],
                                         AF.Sigmoid)
                    nc.scalar.activation(s_t[:, 256:512], ps[:, 256:512],
                                         AF.Sigmoid)
                    sg = s_t[:].rearrange("p (g r) -> p g r", g=4)
                    s_cc, s_i, s_f, s_o = (sg[:, g] for g in range(4))

                    q_t = stp.tile([128, B], F32, tag=f"q{u}", name=f"q{u}")
                    cf_t = stp.tile([128, B], F32, tag=f"cf{u}",
                                    name=f"cf{u}")
                    nc.vector.scalar_tensor_tensor(
                        q_t[:], s_cc, 0.5, s_i,
                        op0=OP.subtract, op1=OP.mult)
                    nc.vector.tensor_tensor(cf_t[:], s_f, c_u[u][:],
                                            op=OP.mult)
                    saved = None
                    if (u, j) in fix_map:
                        r = fix_map[(u, j)]
                        csave = stp.tile([128, B], F32, tag="csave")
                        hsave = stp.tile([128, B], BF16, tag="hsave")
                        nc.vector.tensor_copy(csave[:], c_u[u][:])
                        nc.vector.tensor_copy(hsave[:], h_u[u][:])
                        saved = (csave, hsave, r)
                    nc.vector.scalar_tensor_tensor(
                        c_u[u][:], q_t[:], 2.0, cf_t[:],
                        op0=OP.mult, op1=OP.add)
                    if saved:
                        csave, hsave, r = saved
                        nc.vector.copy_predicated(
                            c_u[u][:], mfix_sb[:, r * B:(r + 1) * B],
                            csave[:])
                    v_t = stp.tile([128, B], F32, tag=f"v{u}", name=f"v{u}")
                    nc.scalar.activation(v_t[:], c_u[u][:], AF.Sigmoid,
                                         scale=2.0)
                    nc.vector.scalar_tensor_tensor(
                        h_u[u][:], v_t[:], 0.5, s_o,
                        op0=OP.subtract, op1=OP.mult)
                    if saved:
                        csave, hsave, r = saved
                        nc.vector.copy_predicated(
                            h_u[u][:], mfix_sb[:, r * B:(r + 1) * B],
                            hsave[:])

                TRPRE = 12  # transpose lookahead (pairs): keeps the
                # in-order tensor queue from stalling on gather completions
                firstneed = {}
                for j in range(L):
                    for u in range(UPC):
                        b = blkmap(u, j)
                        if b not in firstneed:
                            firstneed[b] = j
                tr_at = {}
                for b, j in firstneed.items():
                    tr_at.setdefault(max(j - TRPRE, -1), []).append(b)
                psq = {u: [] for u in range(UPC)}
                for b in sorted(tr_at.get(-1, []),
                                key=lambda b: firstneed[b]):
                    emit_tr(b)
                for u in range(UPC):
                    psq[u].append(emit_wx(u, 0))
                for j in range(L):
                    for b in tr_at.get(j, []):
                        emit_tr(b)
                    for u in range(UPC):
                        if j + 1 < L:
                            psq[u].append(emit_wx(u, j + 1))
                        recurrent_step(u, j, psq[u][j])

                # ---- exchange final states (single masked AllReduce) ----
                hcat = sp.tile([H, 2 * B], F32, tag="hcat")
                for d2 in range(2):
                    dst = hcat[:, d2 * B:(d2 + 1) * B]
                    nc.vector.tensor_scalar(
                        dst, h_u[0][:], sel_sb[:, d2:d2 + 1], None,
                        op0=OP.mult)
                    for u in range(1, UPC):
                        nc.vector.scalar_tensor_tensor(
                            dst, h_u[u][:],
                            sel_sb[:, 2 * u + d2:2 * u + d2 + 1], dst,
                            op0=OP.mult, op1=OP.add)
                cc_in = dp.tile([H, 2 * B], F32, tag="cci")
                cc_out = dp.tile([H, 2 * B], F32, tag="cco")
                nc.sync.dma_start(cc_in[:, :], hcat[:])
                nc.gpsimd.collective_compute(
                    "AllReduce", OP.add,
                    replica_groups=[list(range(NCORES))],
                    ins=[cc_in.opt()], outs=[cc_out.opt()])
                hfull = sp.tile([H, 2 * B], F32, tag="hfull")
                nc.sync.dma_start(hfull[:], cc_out[:, :])

                # ---- BN2 fold + dense + softmax (all 128 examples) ----
                st2 = sp.tile([H, 12], F32, tag="st2")
                scr2 = sp.tile([H, B], F32, tag="scr2")
                hn = sp.tile([H, 2 * B], BF16, tag="hn")
                for d2 in range(2):
                    hd = hfull[:, d2 * B:(d2 + 1) * B]
                    nc.vector.tensor_reduce(st2[:, 0:1], hd,
                                            axis=AX.X, op=OP.add)
                    nc.scalar.activation(scr2[:], hd, AF.Square,
                                         accum_out=st2[:, 1:2])
                    m2 = st2[:, 8:9]
                    v2 = st2[:, 9:10]
                    a2 = st2[:, 10:11]
                    of2 = st2[:, 11:12]
                    nc.vector.tensor_scalar(m2, st2[:, 0:1], 1.0 / B, None,
                                            op0=OP.mult)
                    nc.vector.tensor_scalar(v2, st2[:, 1:2], 1.0 / B, None,
                                            op0=OP.mult)
                    nc.vector.tensor_tensor(a2, m2, m2, op=OP.mult)
                    nc.vector.tensor_tensor(v2, v2, a2, op=OP.subtract)
                    # h halved: eps/4 reproduces BN(h) with eps exactly
                    nc.vector.tensor_scalar(v2, v2, BN_EPS / 4.0, None,
                                            op0=OP.add)
                    nc.scalar.activation(v2, v2, AF.Sqrt)
                    nc.vector.reciprocal(v2, v2)
                    nc.vector.tensor_tensor(a2, g2_sb[:, d2:d2 + 1], v2,
                                            op=OP.mult)
                    nc.vector.tensor_tensor(of2, a2, m2, op=OP.mult)
                    nc.vector.tensor_tensor(of2, be2_sb[:, d2:d2 + 1], of2,
                                            op=OP.subtract)
                    nc.vector.tensor_scalar(hn[:, d2 * B:(d2 + 1) * B], hd,
                                            a2, of2, op0=OP.mult, op1=OP.add)

                ps_o = po.tile([B, ODIM], F32, space="PSUM")
                nc.tensor.matmul(ps_o[:], hn[:, 0:B], wdq[0][:],
                                 start=True, stop=False,
                                 skip_group_check=True)
                nc.tensor.matmul(ps_o[:], hn[:, B:2 * B], wdq[1][:],
                                 start=False, stop=True,
                                 skip_group_check=True)
                z = sp.tile([B, ODIM], F32, tag="z")
                ez = sp.tile([B, ODIM], F32, tag="ez")
                mx = sp.tile([B, 2], F32, tag="mx")
                nc.vector.tensor_tensor(z[:], ps_o[:], bd_sb[:], op=OP.add)
                nc.vector.tensor_reduce(mx[:, 0:1], z[:], axis=AX.X,
                                        op=OP.max)
                nc.vector.tensor_scalar(mx[:, 1:2], mx[:, 0:1], -1.0, None,
                                        op0=OP.mult)
                nc.scalar.activation(ez[:], z[:], AF.Exp, bias=mx[:, 1:2],
                                     accum_out=mx[:, 0:1])
                nc.vector.reciprocal(mx[:, 0:1], mx[:, 0:1])
                nc.vector.tensor_scalar(z[:], ez[:], mx[:, 0:1], None,
                                        op0=OP.mult)
                nc.sync.dma_start(out_d[:, :], z[:])

    nc.finalize()
    return nc


def _block_tokens(core):
    """block index -> token time t for this core (or None for dummy)."""
    toks = []
    for b in range(SEG):
        toks.append(SEG * core + b)
    for i in range(WARM):                       # fwd warmup, ascending t
        t = SEG * core - WARM + i
        toks.append(t if t >= 0 else None)
    for i in range(WARM):                       # bwd warmup, descending t
        t = SEG * (core + 1) + WARM - 1 - i
        toks.append(t if t < T else None)
    return toks


def _core_ids(inputs, core):
    ids = np.asarray(inputs["ids"]).astype(np.int64)  # [B, T]
    out = np.zeros((B, NBLK), np.int64)
    for b, t in enumerate(_block_tokens(core)):
        if t is not None:
            out[:, b] = ids[:, t]
    return out


def _fold_weights(inputs):
    ids = np.asarray(inputs["ids"]).astype(np.int64)
    emb = np.asarray(inputs["embed_table"], np.float64)
    counts = np.bincount(ids.ravel(), minlength=VOCAB).astype(np.float64)
    n = float(B * T)
    sum_x = counts @ emb
    sumsq_x = counts @ (emb * emb)
    m1 = sum_x / n
    v1 = sumsq_x / n - m1 * m1
    a1 = np.asarray(inputs["gamma1"], np.float64) / np.sqrt(v1 + BN_EPS)
    cvec = np.asarray(inputs["beta1"], np.float64) - a1 * m1

    folded = {}
    for d, (wk, uk, bk) in enumerate([("Wf", "Uf", "bf"), ("Wb", "Ub", "bb")]):
        W = np.asarray(inputs[wk], np.float64)
        U = np.asarray(inputs[uk], np.float64)
        b = np.asarray(inputs[bk], np.float64)
        Wp = W * a1[:, None]
        bp = b + cvec @ W
        Up = U * 2.0
        Wp[:, 256:384] *= 2.0
        Up[:, 256:384] *= 2.0
        bp[256:384] *= 2.0
        # permute gate blocks [i,f,cc,o] -> [cc,i,f,o] (split-sigmoid order)
        perm = [2, 0, 1, 3]
        Wp = np.concatenate([Wp[:, 128 * p:128 * (p + 1)] for p in perm],
                            axis=1)
        Up = np.concatenate([Up[:, 128 * p:128 * (p + 1)] for p in perm],
                            axis=1)
        bp = np.concatenate([bp[128 * p:128 * (p + 1)] for p in perm])
        folded[d] = (
            np.ascontiguousarray(
                Wp.astype(np.float32).astype(ml_dtypes.bfloat16)),
            np.ascontiguousarray(
                Up.astype(np.float32).astype(ml_dtypes.bfloat16)),
            np.ascontiguousarray(
                bp.astype(np.float32).astype(ml_dtypes.bfloat16)
                .reshape(4, 128)),
        )
    return folded


def kernel(**inputs):
    global LAST_RESULT

    core_ids = [_core_ids(inputs, c) for c in range(NCORES)]
    sched = set()
    for c in range(NCORES):
        for u in range(UPC):
            for j in range(L):
                if (core_ids[c][:, blkmap(u, j)] == 0).any():
                    sched.add((u, int(j)))
    mask_sched = sorted(sched)
    NFIX = len(mask_sched)

    nc = build_program(mask_sched)

    folded = _fold_weights(inputs)
    emb_bf = np.ascontiguousarray(
        np.asarray(inputs["embed_table"], np.float32)
        .astype(ml_dtypes.bfloat16))
    Wd = np.asarray(inputs["Wd"], np.float32)
    com = {
        "emb": emb_bf,
        "g2": np.ascontiguousarray(
            np.asarray(inputs["gamma2"], np.float32).reshape(2, H).T),
        "be2": np.ascontiguousarray(
            np.asarray(inputs["beta2"], np.float32).reshape(2, H).T),
        "Wd0": np.ascontiguousarray(Wd[0:H, :].astype(ml_dtypes.bfloat16)),
        "Wd1": np.ascontiguousarray(
            Wd[H:2 * H, :].astype(ml_dtypes.bfloat16)),
        "bd": np.ascontiguousarray(
            np.broadcast_to(np.asarray(inputs["bd"], np.float32),
                            (B, ODIM))),
    }

    in_maps = []
    for c in range(NCORES):
        m = dict(com)
        m["ids"] = np.ascontiguousarray(core_ids[c].astype(np.int32))
        sel = np.zeros((H, 2 * UPC), np.float32)
        if c == NCORES - 1:
            sel[:, 0] = 1.0        # fwd final lives on core 7 (slot 0)
        if c == 0:
            sel[:, 3] = 1.0        # bwd final lives on core 0 (slot 1)
        for u in range(UPC):
            W_b, U_b, Bp_b = folded[u]   # slot 0 = fwd dir, slot 1 = bwd
            m[f"W{u}"] = W_b
            m[f"U{u}"] = U_b
            m[f"Bp{u}"] = Bp_b
        m["sel"] = sel
        if NFIX:
            mf = np.zeros((NFIX, 128, B), np.uint8)
            for r, (u, j) in enumerate(mask_sched):
                inv = (core_ids[c][:, blkmap(u, j)] == 0).astype(np.uint8)
                mf[r, :, :] = inv[None, :]
            m["mfix"] = mf.reshape(NFIX * 128, B)
        in_maps.append(m)

    res = run_bass_kernel_spmd(nc, in_maps, list(range(NCORES)),
                               trace=TRACE, tmpdir=TRACE_DIR)
    LAST_RESULT = {"exec_time_ns": res.exec_time_ns}
    return np.asarray(res.results[0]["out"]).astype(np.float32)


# revision 30
# speedup vs baseline: 1.0057x; 1.0057x over previous
"""Trainium2 Bass kernel for BiLSTM text classifier — 16-way time-split.

16 units = 2 directions x 8 time segments; each of the 8 cores interleaves
TWO units (one per direction) step by step, so the two dependency chains
hide each other's latency and the core becomes throughput-bound instead of
chain-bound.  Each unit runs a 156-step window (124-156 real + 32 warmup)
over ALL 128 examples.  A 32-step zero-state warmup reproduces the true
LSTM state to ~2e-7 (forget gates average ~sigmoid(+-1): state decays ~2x
per step).

BN1 is folded into the input weights on the host (exact batch stats via
bincount).  h is stored halved (h' = sig_o*(sig(2c)-0.5)) with the 2x
folded into U; BN2 consumes h' with eps/4.  Gates are host-permuted to
[cc,i,f,o] for a split sigmoid.  The embedding gather is pipelined into
the scan (indirect DMA), with the x-transpose on the PE.  Final states are
exchanged with one masked AllReduce; every core then computes BN2 + dense
+ softmax for the full batch and the host reads core 0's output.
"""

import os
import sys

sys.path.insert(0, "/opt/trn_rl_repo")

import ml_dtypes
import numpy as np

from concourse import bacc, bass, mybir, tile
from concourse.bass import IndirectOffsetOnAxis
from concourse.bass_utils import run_bass_kernel_spmd
from concourse.masks import make_identity

F32 = mybir.dt.float32
BF16 = mybir.dt.bfloat16
I32 = mybir.dt.int32
AF = mybir.ActivationFunctionType
OP = mybir.AluOpType
AX = mybir.AxisListType

# Problem dims
B, T, E, H, ODIM, VOCAB = 128, 1024, 128, 128, 10, 100000
G4 = 4 * H  # 512
NCORES = 8
UPC = 2                      # units per core (slot 0 = fwd, slot 1 = bwd)
NSEGD = NCORES               # 8 equal segments, one per core
SEG = T // NSEGD             # 128
WARM = 24
L = SEG + WARM               # 152 steps per unit
# x_T block layout per core: [0..SEG) shared real blocks (block b = step
# t=SEG*c+b), [SEG..SEG+WARM) fwd warmup, [SEG+WARM..SEG+2W) bwd warmup.
# Core c runs fwd AND bwd over ITS OWN segment, so the real blocks are
# gathered once and consumed by both units (bwd reads them in reverse).
# Edge warmups (fwd on core 0, bwd on core 7) use id=0 dummy blocks whose
# all-masked steps carry the exact zero initial state.
NBLK = SEG + 2 * WARM        # 176 gathered blocks per core


def blkmap(u, j):
    if u == 0:  # fwd
        return SEG + j if j < WARM else j - WARM
    return SEG + WARM + j if j < WARM else SEG - 1 - (j - WARM)
BN_EPS = 1e-3

TRACE = False
TRACE_DIR = None
LAST_RESULT = {}


def build_program(mask_sched):
    """mask_sched: sorted list of (slot, local step) pairs (union over
    cores) needing masked-carry fixups; per-core data arrives via 'mfix'."""
    nc = bacc.Bacc("TRN2", target_bir_lowering=False, debug=False,
                   num_devices=NCORES)

    NFIX = len(mask_sched)

    ids_d = nc.dram_tensor("ids", [128, NBLK], I32, kind="ExternalInput")
    emb_d = nc.dram_tensor("emb", [VOCAB, E], BF16, kind="ExternalInput")
    W_d = [nc.dram_tensor(f"W{u}", [E, G4], BF16, kind="ExternalInput")
           for u in range(UPC)]
    U_d = [nc.dram_tensor(f"U{u}", [H, G4], BF16, kind="ExternalInput")
           for u in range(UPC)]
    Bp_d = [nc.dram_tensor(f"Bp{u}", [4, 128], BF16, kind="ExternalInput")
            for u in range(UPC)]
    sel_d = nc.dram_tensor("sel", [H, 2 * UPC], F32, kind="ExternalInput")
    g2_d = nc.dram_tensor("g2", [H, 2], F32, kind="ExternalInput")
    be2_d = nc.dram_tensor("be2", [H, 2], F32, kind="ExternalInput")
    Wd0_d = nc.dram_tensor("Wd0", [H, ODIM], BF16, kind="ExternalInput")
    Wd1_d = nc.dram_tensor("Wd1", [H, ODIM], BF16, kind="ExternalInput")
    bd_d = nc.dram_tensor("bd", [B, ODIM], F32, kind="ExternalInput")
    if NFIX:
        mfix_d = nc.dram_tensor("mfix", [NFIX * 128, B], mybir.dt.uint8,
                                kind="ExternalInput")
    out_d = nc.dram_tensor("out", [B, ODIM], F32, kind="ExternalOutput")

    with tile.TileContext(nc) as tc:
        with (
            tc.tile_pool(name="const", bufs=1) as cp,
            tc.tile_pool(name="xt", bufs=1) as xp,
            tc.tile_pool(name="state", bufs=1) as sp,
            tc.tile_pool(name="step", bufs=2) as stp,
            tc.tile_pool(name="nat", bufs=64) as natp,
            tc.tile_pool(name="dram", bufs=1, space="DRAM") as dp,
        ):
            ids_sb = cp.tile([128, NBLK], I32)
            x_T = xp.tile([E, NBLK * 128], BF16)
            wq = [cp.tile([E, G4], BF16, tag=f"w{u}", name=f"w{u}")
                  for u in range(UPC)]
            uq = [cp.tile([H, G4], BF16, tag=f"u{u}", name=f"u{u}")
                  for u in range(UPC)]
            Bp = [cp.tile([4, 128], BF16, tag=f"Bp{u}", name=f"Bp{u}")
                  for u in range(UPC)]
            Gind = cp.tile([4, G4], BF16)
            sel_sb = cp.tile([H, 2 * UPC], F32)
            wdq = [cp.tile([H, ODIM], BF16, tag=f"wd{d}", name=f"wd{d}")
                   for d in range(2)]
            bd_sb = cp.tile([B, ODIM], F32)
            g2_sb = cp.tile([H, 2], F32)
            be2_sb = cp.tile([H, 2], F32)
            if NFIX:
                mfix_sb = cp.tile([128, NFIX * B], mybir.dt.uint8)

            h_u = [sp.tile([H, B], BF16, tag=f"h{u}", name=f"h{u}")
                   for u in range(UPC)]
            c_u = [sp.tile([H, B], F32, tag=f"c{u}", name=f"c{u}")
                   for u in range(UPC)]
            ident = cp.tile([128, 128], BF16)
            make_identity(nc, ident[:])

            nc.sync.dma_start(ids_sb[:], ids_d[:, :])
            for u in range(UPC):
                nc.sync.dma_start(wq[u][:], W_d[u][:, :])
                nc.sync.dma_start(uq[u][:], U_d[u][:, :])
                nc.sync.dma_start(Bp[u][:], Bp_d[u][:, :])
            nc.sync.dma_start(sel_sb[:], sel_d[:, :])
            nc.sync.dma_start(wdq[0][:], Wd0_d[:, :])
            nc.sync.dma_start(wdq[1][:], Wd1_d[:, :])
            nc.sync.dma_start(bd_sb[:], bd_d[:, :])
            nc.sync.dma_start(g2_sb[:], g2_d[:, :])
            nc.sync.dma_start(be2_sb[:], be2_d[:, :])
            if NFIX:
                for r in range(NFIX):
                    nc.sync.dma_start(
                        mfix_sb[:, r * B:(r + 1) * B],
                        mfix_d[r * 128:(r + 1) * 128, :])
            for u in range(UPC):
                nc.vector.memset(h_u[u][:], 0.0)
                nc.vector.memset(c_u[u][:], 0.0)

            # gate-block indicator for the rank-4 bias matmul
            nc.gpsimd.memset(Gind[:], 0.0)
            nc.gpsimd.affine_select(
                out=Gind[:].rearrange("p (q r) -> p q r", q=4),
                in_=Gind[:].rearrange("p (q r) -> p q r", q=4),
                compare_op=OP.not_equal,
                fill=1.0,
                base=0,
                pattern=[[1, 4], [0, 128]],
                channel_multiplier=-1,
            )

            # ---- gather (consumption order), pipelined into the scan ----
            gorder = []
            for i in range(WARM):                 # warmups first
                gorder += [SEG + i, SEG + WARM + i]
            for b in range(SEG // 2):             # shared from both ends
                gorder += [b, SEG - 1 - b]
            xnats = {}
            for blk in gorder:
                xnat = natp.tile([128, E], BF16, tag="xnat", name="xnat")
                nc.gpsimd.indirect_dma_start(
                    out=xnat[:],
                    out_offset=None,
                    in_=emb_d[:, :],
                    in_offset=IndirectOffsetOnAxis(
                        ap=ids_sb[:, blk:blk + 1], axis=0),
                )
                xnats[blk] = xnat

            fix_map = {}
            for r, key in enumerate(mask_sched):
                fix_map[tuple(key)] = r

            # ---- the interleaved scan ----
            with (
                tc.tile_pool(name="pstep", bufs=4, space="PSUM") as pstep,
                tc.tile_pool(name="pso", bufs=1, space="PSUM") as po,
                tc.tile_pool(name="ptr", bufs=3, space="PSUM") as ptrp,
            ):
                def emit_tr(blk):
                    pt = ptrp.tile([128, 128], BF16, space="PSUM", tag="pt",
                                   name="pt")
                    nc.tensor.transpose(pt[:], xnats[blk][:], ident[:])
                    col = blk * 128
                    nc.vector.tensor_copy(x_T[:, col:col + 128], pt[:])

                def emit_wx(u, j):
                    ps = pstep.tile([128, G4], F32, space="PSUM", tag="ps",
                                    name="ps")
                    col = blkmap(u, j) * 128
                    toks = x_T[:, col:col + 128]
                    for g in range(4):
                        nc.tensor.matmul(
                            ps[:, g * 128:(g + 1) * 128],
                            wq[u][:, g * 128:(g + 1) * 128], toks,
                            start=(g == 0), stop=False,
                            skip_group_check=True)
                    nc.tensor.matmul(ps[:], Bp[u][:], Gind[:],
                                     start=False, stop=False,
                                     skip_group_check=True)
                    return ps

                def recurrent_step(u, j, ps):
                    for g in range(4):
                        nc.tensor.matmul(
                            ps[:, g * 128:(g + 1) * 128],
                            uq[u][:, g * 128:(g + 1) * 128], h_u[u][:],
                            start=False, stop=True,
                            skip_group_check=True)
                    s_t = stp.tile([128, G4], F32, tag=f"s{u}",
                                   name=f"s{u}")
                    nc.scalar.activation(s_t[:, 0:256], ps[:, 0:256],
                                         AF.Sigmoid)
                    nc.scalar.activation(s_t[:, 256:512], ps[:, 256:512],
                                         AF.Sigmoid)
                    sg = s_t[:].rearrange("p (g r) -> p g r", g=4)
                    s_cc, s_i, s_f, s_o = (sg[:, g] for g in range(4))

                    q_t = stp.tile([128, B], F32, tag=f"q{u}", name=f"q{u}")
                    cf_t = stp.tile([128, B], F32, tag=f"cf{u}",
                                    name=f"cf{u}")
                    nc.vector.scalar_tensor_tensor(
                        q_t[:], s_cc, 0.5, s_i,
                        op0=OP.subtract, op1=OP.mult)
                    nc.vector.tensor_tensor(cf_t[:], s_f, c_u[u][:],
                                            op=OP.mult)
                    saved = None
                    if (u, j) in fix_map:
                        r = fix_map[(u, j)]
                        csave = stp.tile([128, B], F32, tag="csave")
                        hsave = stp.tile([128, B], BF16, tag="hsave")
                        nc.vector.tensor_copy(csave[:], c_u[u][:])
                        nc.vector.tensor_copy(hsave[:], h_u[u][:])
                        saved = (csave, hsave, r)
                    nc.vector.scalar_tensor_tensor(
                        c_u[u][:], q_t[:], 2.0, cf_t[:],
                        op0=OP.mult, op1=OP.add)
                    if saved:
                        csave, hsave, r = saved
                        nc.vector.copy_predicated(
                            c_u[u][:], mfix_sb[:, r * B:(r + 1) * B],
                            csave[:])
                    v_t = stp.tile([128, B], F32, tag=f"v{u}", name=f"v{u}")
                    nc.scalar.activation(v_t[:], c_u[u][:], AF.Sigmoid,
                                         scale=2.0)
                    nc.vector.scalar_tensor_tensor(
                        h_u[u][:], v_t[:], 0.5, s_o,
                        op0=OP.subtract, op1=OP.mult)
                    if saved:
                        csave, hsave, r = saved
                        nc.vector.copy_predicated(
                            h_u[u][:], mfix_sb[:, r * B:(r + 1) * B],
                            hsave[:])

                TRPRE = 12  # transpose lookahead (pairs): keeps the
                # in-order tensor queue from stalling on gather completions
                firstneed = {}
                for j in range(L):
                    for u in range(UPC):
                        b = blkmap(u, j)
                        if b not in firstneed:
                            firstneed[b] = j
                tr_at = {}
                for b, j in firstneed.items():
                    tr_at.setdefault(max(j - TRPRE, -1), []).append(b)
                psq = {u: [] for u in range(UPC)}
                for b in sorted(tr_at.get(-1, []),
                                key=lambda b: firstneed[b]):
                    emit_tr(b)
                for u in range(UPC):
                    psq[u].append(emit_wx(u, 0))
                for j in range(L):
                    for b in tr_at.get(j, []):
                        emit_tr(b)
                    for u in range(UPC):
                        if j + 1 < L:
                            psq[u].append(emit_wx(u, j + 1))
                        recurrent_step(u, j, psq[u][j])

                # ---- exchange final states (single masked AllReduce) ----
                hcat = sp.tile([H, 2 * B], F32, tag="hcat")
                for d2 in range(2):
                    dst = hcat[:, d2 * B:(d2 + 1) * B]
                    nc.vector.tensor_scalar(
                        dst, h_u[0][:], sel_sb[:, d2:d2 + 1], None,
                        op0=OP.mult)
                    for u in range(1, UPC):
                        nc.vector.scalar_tensor_tensor(
                            dst, h_u[u][:],
                            sel_sb[:, 2 * u + d2:2 * u + d2 + 1], dst,
                            op0=OP.mult, op1=OP.add)
                cc_in = dp.tile([H, 2 * B], F32, tag="cci")
                cc_out = dp.tile([H, 2 * B], F32, tag="cco")
                nc.sync.dma_start(cc_in[:, :], hcat[:])
                nc.gpsimd.collective_compute(
                    "AllReduce", OP.add,
                    replica_groups=[list(range(NCORES))],
                    ins=[cc_in.opt()], outs=[cc_out.opt()])
                hfull = sp.tile([H, 2 * B], F32, tag="hfull")
                nc.sync.dma_start(hfull[:], cc_out[:, :])

                # ---- BN2 fold + dense + softmax (all 128 examples) ----
                st2 = sp.tile([H, 12], F32, tag="st2")
                scr2 = sp.tile([H, B], F32, tag="scr2")
                hn = sp.tile([H, 2 * B], BF16, tag="hn")
                for d2 in range(2):
                    hd = hfull[:, d2 * B:(d2 + 1) * B]
                    nc.vector.tensor_reduce(st2[:, 0:1], hd,
                                            axis=AX.X, op=OP.add)
                    nc.scalar.activation(scr2[:], hd, AF.Square,
                                         accum_out=st2[:, 1:2])
                    m2 = st2[:, 8:9]
                    v2 = st2[:, 9:10]
                    a2 = st2[:, 10:11]
                    of2 = st2[:, 11:12]
                    nc.vector.tensor_scalar(m2, st2[:, 0:1], 1.0 / B, None,
                                            op0=OP.mult)
                    nc.vector.tensor_scalar(v2, st2[:, 1:2], 1.0 / B, None,
                                            op0=OP.mult)
                    nc.vector.tensor_tensor(a2, m2, m2, op=OP.mult)
                    nc.vector.tensor_tensor(v2, v2, a2, op=OP.subtract)
                    # h halved: eps/4 reproduces BN(h) with eps exactly
                    nc.vector.tensor_scalar(v2, v2, BN_EPS / 4.0, None,
                                            op0=OP.add)
                    nc.scalar.activation(v2, v2, AF.Sqrt)
                    nc.vector.reciprocal(v2, v2)
                    nc.vector.tensor_tensor(a2, g2_sb[:, d2:d2 + 1], v2,
                                            op=OP.mult)
                    nc.vector.tensor_tensor(of2, a2, m2, op=OP.mult)
                    nc.vector.tensor_tensor(of2, be2_sb[:, d2:d2 + 1], of2,
                                            op=OP.subtract)
                    nc.vector.tensor_scalar(hn[:, d2 * B:(d2 + 1) * B], hd,
                                            a2, of2, op0=OP.mult, op1=OP.add)

                ps_o = po.tile([B, ODIM], F32, space="PSUM")
                nc.tensor.matmul(ps_o[:], hn[:, 0:B], wdq[0][:],
                                 start=True, stop=False,
                                 skip_group_check=True)
                nc.tensor.matmul(ps_o[:], hn[:, B:2 * B], wdq[1][:],
                                 start=False, stop=True,
                                 skip_group_check=True)
                z = sp.tile([B, ODIM], F32, tag="z")
                ez = sp.tile([B, ODIM], F32, tag="ez")
                mx = sp.tile([B, 2], F32, tag="mx")
                nc.vector.tensor_tensor(z[:], ps_o[:], bd_sb[:], op=OP.add)
                nc.vector.tensor_reduce(mx[:, 0:1], z[:], axis=AX.X,
                                        op=OP.max)
                nc.vector.tensor_scalar(mx[:, 1:2], mx[:, 0:1], -1.0, None,
                                        op0=OP.mult)
                nc.scalar.activation(ez[:], z[:], AF.Exp, bias=mx[:, 1:2],
                                     accum_out=mx[:, 0:1])
                nc.vector.reciprocal(mx[:, 0:1], mx[:, 0:1])
                nc.vector.tensor_scalar(z[:], ez[:], mx[:, 0:1], None,
                                        op0=OP.mult)
                nc.sync.dma_start(out_d[:, :], z[:])

    nc.finalize()
    return nc


def _block_tokens(core):
    """block index -> token time t for this core (or None for dummy)."""
    toks = []
    for b in range(SEG):
        toks.append(SEG * core + b)
    for i in range(WARM):                       # fwd warmup, ascending t
        t = SEG * core - WARM + i
        toks.append(t if t >= 0 else None)
    for i in range(WARM):                       # bwd warmup, descending t
        t = SEG * (core + 1) + WARM - 1 - i
        toks.append(t if t < T else None)
    return toks


def _core_ids(inputs, core):
    ids = np.asarray(inputs["ids"]).astype(np.int64)  # [B, T]
    out = np.zeros((B, NBLK), np.int64)
    for b, t in enumerate(_block_tokens(core)):
        if t is not None:
            out[:, b] = ids[:, t]
    return out


def _fold_weights(inputs):
    ids = np.asarray(inputs["ids"]).astype(np.int64)
    emb = np.asarray(inputs["embed_table"], np.float64)
    counts = np.bincount(ids.ravel(), minlength=VOCAB).astype(np.float64)
    n = float(B * T)
    sum_x = counts @ emb
    sumsq_x = counts @ (emb * emb)
    m1 = sum_x / n
    v1 = sumsq_x / n - m1 * m1
    a1 = np.asarray(inputs["gamma1"], np.float64) / np.sqrt(v1 + BN_EPS)
    cvec = np.asarray(inputs["beta1"], np.float64) - a1 * m1

    folded = {}
    for d, (wk, uk, bk) in enumerate([("Wf", "Uf", "bf"), ("Wb", "Ub", "bb")]):
        W = np.asarray(inputs[wk], np.float64)
        U = np.asarray(inputs[uk], np.float64)
        b = np.asarray(inputs[bk], np.float64)
        Wp = W * a1[:, None]
        bp = b + cvec @ W
        Up = U * 2.0
        Wp[:, 256:384] *= 2.0
        Up[:, 256:384] *= 2.0
        bp[256:384] *= 2.0
        # permute gate blocks [i,f,cc,o] -> [cc,i,f,o] (split-sigmoid order)
        perm = [2, 0, 1, 3]
        Wp = np.concatenate([Wp[:, 128 * p:128 * (p + 1)] for p in perm],
                            axis=1)
        Up = np.concatenate([Up[:, 128 * p:128 * (p + 1)] for p in perm],
                            axis=1)
        bp = np.concatenate([bp[128 * p:128 * (p + 1)] for p in perm])
        folded[d] = (
            np.ascontiguousarray(
                Wp.astype(np.float32).astype(ml_dtypes.bfloat16)),
            np.ascontiguousarray(
                Up.astype(np.float32).astype(ml_dtypes.bfloat16)),
            np.ascontiguousarray(
                bp.astype(np.float32).astype(ml_dtypes.bfloat16)
                .reshape(4, 128)),
        )
    return folded


def kernel(**inputs):
    global LAST_RESULT

    core_ids = [_core_ids(inputs, c) for c in range(NCORES)]
    sched = set()
    for c in range(NCORES):
        for u in range(UPC):
            for j in range(L):
                if (core_ids[c][:, blkmap(u, j)] == 0).any():
                    sched.add((u, int(j)))
    mask_sched = sorted(sched)
    NFIX = len(mask_sched)

    nc = build_program(mask_sched)

    folded = _fold_weights(inputs)
    emb_bf = np.ascontiguousarray(
        np.asarray(inputs["embed_table"], np.float32)
        .astype(ml_dtypes.bfloat16))
    Wd = np.asarray(inputs["Wd"], np.float32)
    com = {
        "emb": emb_bf,
        "g2": np.ascontiguousarray(
            np.asarray(inputs["gamma2"], np.float32).reshape(2, H).T),
        "be2": np.ascontiguousarray(
            np.asarray(inputs["beta2"], np.float32).reshape(2, H).T),
        "Wd0": np.ascontiguousarray(Wd[0:H, :].astype(ml_dtypes.bfloat16)),
        "Wd1": np.ascontiguousarray(
            Wd[H:2 * H, :].astype(ml_dtypes.bfloat16)),
        "bd": np.ascontiguousarray(
            np.broadcast_to(np.asarray(inputs["bd"], np.float32),
                            (B, ODIM))),
    }

    in_maps = []
    for c in range(NCORES):
        m = dict(com)
        m["ids"] = np.ascontiguousarray(core_ids[c].astype(np.int32))
        sel = np.zeros((H, 2 * UPC), np.float32)
        if c == NCORES - 1:
            sel[:, 0] = 1.0        # fwd final lives on core 7 (slot 0)
        if c == 0:
            sel[:, 3] = 1.0        # bwd final lives on core 0 (slot 1)
        for u in range(UPC):
            W_b, U_b, Bp_b = folded[u]   # slot 0 = fwd dir, slot 1 = bwd
            m[f"W{u}"] = W_b
            m[f"U{u}"] = U_b
            m[f"Bp{u}"] = Bp_b
        m["sel"] = sel
        if NFIX:
            mf = np.zeros((NFIX, 128, B), np.uint8)
            for r, (u, j) in enumerate(mask_sched):
                inv = (core_ids[c][:, blkmap(u, j)] == 0).astype(np.uint8)
                mf[r, :, :] = inv[None, :]
            m["mfix"] = mf.reshape(NFIX * 128, B)
        in_maps.append(m)

    res = run_bass_kernel_spmd(nc, in_maps, list(range(NCORES)),
                               trace=TRACE, tmpdir=TRACE_DIR)
    LAST_RESULT = {"exec_time_ns": res.exec_time_ns}
    return np.asarray(res.results[0]["out"]).astype(np.float32)


# revision 31
# speedup vs baseline: 1.1468x; 1.1404x over previous
"""Trainium2 Bass kernel for BiLSTM text classifier — 16-way time-split.

16 units = 2 directions x 8 time segments; each of the 8 cores interleaves
TWO units (one per direction) step by step, so the two dependency chains
hide each other's latency and the core becomes throughput-bound instead of
chain-bound.  Each unit runs a 156-step window (124-156 real + 32 warmup)
over ALL 128 examples.  A 32-step zero-state warmup reproduces the true
LSTM state to ~2e-7 (forget gates average ~sigmoid(+-1): state decays ~2x
per step).

BN1 is folded into the input weights on the host (exact batch stats via
bincount).  h is stored halved (h' = sig_o*(sig(2c)-0.5)) with the 2x
folded into U; BN2 consumes h' with eps/4.  Gates are host-permuted to
[cc,i,f,o] for a split sigmoid.  The embedding gather is pipelined into
the scan (indirect DMA), with the x-transpose on the PE.  Final states are
exchanged with one masked AllReduce; every core then computes BN2 + dense
+ softmax for the full batch and the host reads core 0's output.
"""

import os
import sys

sys.path.insert(0, "/opt/trn_rl_repo")

import ml_dtypes
import numpy as np

from concourse import bacc, bass, mybir, tile
from concourse.bass import IndirectOffsetOnAxis
from concourse.bass_utils import run_bass_kernel_spmd
from concourse.masks import make_identity

F32 = mybir.dt.float32
BF16 = mybir.dt.bfloat16
I32 = mybir.dt.int32
AF = mybir.ActivationFunctionType
OP = mybir.AluOpType
AX = mybir.AxisListType

# Problem dims
B, T, E, H, ODIM, VOCAB = 128, 1024, 128, 128, 10, 100000
G4 = 4 * H  # 512
NCORES = 8
UPC = 2                      # units per core (slot 0 = fwd, slot 1 = bwd)
NSEGD = NCORES               # 8 equal segments, one per core
SEG = T // NSEGD             # 128
WARM = 24
L = SEG + WARM               # 152 steps per unit
# x_T block layout per core: [0..SEG) shared real blocks (block b = step
# t=SEG*c+b), [SEG..SEG+WARM) fwd warmup, [SEG+WARM..SEG+2W) bwd warmup.
# Core c runs fwd AND bwd over ITS OWN segment, so the real blocks are
# gathered once and consumed by both units (bwd reads them in reverse).
# Edge warmups (fwd on core 0, bwd on core 7) use id=0 dummy blocks whose
# all-masked steps carry the exact zero initial state.
NBLK = SEG + 2 * WARM        # 176 gathered blocks per core


def blkmap(u, j):
    if u == 0:  # fwd
        return SEG + j if j < WARM else j - WARM
    return SEG + WARM + j if j < WARM else SEG - 1 - (j - WARM)
BN_EPS = 1e-3

TRACE = False
TRACE_DIR = None
LAST_RESULT = {}


def build_program(mask_sched):
    """mask_sched: sorted list of (slot, local step) pairs (union over
    cores) needing masked-carry fixups; per-core data arrives via 'mfix'."""
    nc = bacc.Bacc("TRN2", target_bir_lowering=False, debug=False,
                   num_devices=NCORES)

    NFIX = len(mask_sched)

    ids_d = nc.dram_tensor("ids", [128, NBLK], I32, kind="ExternalInput")
    emb_d = nc.dram_tensor("emb", [VOCAB, E], BF16, kind="ExternalInput")
    W_d = [nc.dram_tensor(f"W{u}", [E, G4], BF16, kind="ExternalInput")
           for u in range(UPC)]
    U_d = [nc.dram_tensor(f"U{u}", [H, G4], BF16, kind="ExternalInput")
           for u in range(UPC)]
    Bp_d = [nc.dram_tensor(f"Bp{u}", [4, 128], BF16, kind="ExternalInput")
            for u in range(UPC)]
    sel_d = nc.dram_tensor("sel", [H, 2 * UPC], F32, kind="ExternalInput")
    g2_d = nc.dram_tensor("g2", [H, 2], F32, kind="ExternalInput")
    be2_d = nc.dram_tensor("be2", [H, 2], F32, kind="ExternalInput")
    Wd0_d = nc.dram_tensor("Wd0", [H, ODIM], BF16, kind="ExternalInput")
    Wd1_d = nc.dram_tensor("Wd1", [H, ODIM], BF16, kind="ExternalInput")
    bd_d = nc.dram_tensor("bd", [B, ODIM], F32, kind="ExternalInput")
    if NFIX:
        mfix_d = nc.dram_tensor("mfix", [NFIX * 128, B], mybir.dt.uint8,
                                kind="ExternalInput")
    out_d = nc.dram_tensor("out", [B, ODIM], F32, kind="ExternalOutput")

    with tile.TileContext(nc) as tc:
        with (
            tc.tile_pool(name="const", bufs=1) as cp,
            tc.tile_pool(name="xt", bufs=1) as xp,
            tc.tile_pool(name="state", bufs=1) as sp,
            tc.tile_pool(name="step", bufs=2) as stp,
            tc.tile_pool(name="nat", bufs=64) as natp,
            tc.tile_pool(name="dram", bufs=1, space="DRAM") as dp,
        ):
            ids_sb = cp.tile([128, NBLK], I32)
            x_T = xp.tile([E, NBLK * 128], BF16)
            wq = [cp.tile([E, G4], BF16, tag=f"w{u}", name=f"w{u}")
                  for u in range(UPC)]
            uq = [cp.tile([H, G4], BF16, tag=f"u{u}", name=f"u{u}")
                  for u in range(UPC)]
            Bp = [cp.tile([4, 128], BF16, tag=f"Bp{u}", name=f"Bp{u}")
                  for u in range(UPC)]
            Gind = cp.tile([4, G4], BF16)
            sel_sb = cp.tile([H, 2 * UPC], F32)
            wdq = [cp.tile([H, ODIM], BF16, tag=f"wd{d}", name=f"wd{d}")
                   for d in range(2)]
            bd_sb = cp.tile([B, ODIM], F32)
            g2_sb = cp.tile([H, 2], F32)
            be2_sb = cp.tile([H, 2], F32)
            if NFIX:
                mfix_sb = cp.tile([128, NFIX * B], mybir.dt.uint8)

            h_u = [sp.tile([H, B], BF16, tag=f"h{u}", name=f"h{u}")
                   for u in range(UPC)]
            c_u = [sp.tile([H, B], F32, tag=f"c{u}", name=f"c{u}")
                   for u in range(UPC)]
            ident = cp.tile([128, 128], BF16)
            make_identity(nc, ident[:])

            nc.sync.dma_start(ids_sb[:], ids_d[:, :])
            for u in range(UPC):
                nc.sync.dma_start(wq[u][:], W_d[u][:, :])
                nc.sync.dma_start(uq[u][:], U_d[u][:, :])
                nc.sync.dma_start(Bp[u][:], Bp_d[u][:, :])
            nc.sync.dma_start(sel_sb[:], sel_d[:, :])
            nc.sync.dma_start(wdq[0][:], Wd0_d[:, :])
            nc.sync.dma_start(wdq[1][:], Wd1_d[:, :])
            nc.sync.dma_start(bd_sb[:], bd_d[:, :])
            nc.sync.dma_start(g2_sb[:], g2_d[:, :])
            nc.sync.dma_start(be2_sb[:], be2_d[:, :])
            if NFIX:
                for r in range(NFIX):
                    nc.sync.dma_start(
                        mfix_sb[:, r * B:(r + 1) * B],
                        mfix_d[r * 128:(r + 1) * 128, :])
            for u in range(UPC):
                nc.vector.memset(h_u[u][:], 0.0)
                nc.vector.memset(c_u[u][:], 0.0)

            # gate-block indicator for the rank-4 bias matmul
            nc.gpsimd.memset(Gind[:], 0.0)
            nc.gpsimd.affine_select(
                out=Gind[:].rearrange("p (q r) -> p q r", q=4),
                in_=Gind[:].rearrange("p (q r) -> p q r", q=4),
                compare_op=OP.not_equal,
                fill=1.0,
                base=0,
                pattern=[[1, 4], [0, 128]],
                channel_multiplier=-1,
            )

            # ---- gather (consumption order), pipelined into the scan ----
            gorder = []
            for i in range(WARM):                 # warmups first
                gorder += [SEG + i, SEG + WARM + i]
            for b in range(SEG // 2):             # shared from both ends
                gorder += [b, SEG - 1 - b]
            xnats = {}
            for blk in gorder:
                xnat = natp.tile([128, E], BF16, tag="xnat", name="xnat")
                nc.gpsimd.indirect_dma_start(
                    out=xnat[:],
                    out_offset=None,
                    in_=emb_d[:, :],
                    in_offset=IndirectOffsetOnAxis(
                        ap=ids_sb[:, blk:blk + 1], axis=0),
                )
                xnats[blk] = xnat

            fix_map = {}
            for r, key in enumerate(mask_sched):
                fix_map[tuple(key)] = r

            # ---- the interleaved scan ----
            with (
                tc.tile_pool(name="pstep", bufs=4, space="PSUM") as pstep,
                tc.tile_pool(name="pso", bufs=1, space="PSUM") as po,
                tc.tile_pool(name="ptr", bufs=2, space="PSUM") as ptrp,
                tc.tile_pool(name="pdum", bufs=1, space="PSUM") as pdum_p,
            ):
                # Write-only dummy matmuls keep the PE activity monitor
                # above its throttle threshold (HAM): without them the
                # array duty cycle is ~35% and the clock stays at 1.2 GHz.
                pdum = pdum_p.tile([128, G4], F32, space="PSUM",
                                   tag="pdum", name="pdum")
                def emit_tr(blk):
                    pt = ptrp.tile([128, 128], BF16, space="PSUM", tag="pt",
                                   name="pt")
                    nc.tensor.transpose(pt[:], xnats[blk][:], ident[:])
                    col = blk * 128
                    nc.vector.tensor_copy(x_T[:, col:col + 128], pt[:])

                def emit_wx(u, j):
                    ps = pstep.tile([128, G4], F32, space="PSUM", tag="ps",
                                    name="ps")
                    col = blkmap(u, j) * 128
                    toks = x_T[:, col:col + 128]
                    for g in range(4):
                        nc.tensor.matmul(
                            ps[:, g * 128:(g + 1) * 128],
                            wq[u][:, g * 128:(g + 1) * 128], toks,
                            start=(g == 0), stop=False,
                            skip_group_check=True)
                    nc.tensor.matmul(ps[:], Bp[u][:], Gind[:],
                                     start=False, stop=False,
                                     skip_group_check=True)
                    return ps

                def recurrent_step(u, j, ps):
                    for g in range(4):
                        nc.tensor.matmul(
                            ps[:, g * 128:(g + 1) * 128],
                            uq[u][:, g * 128:(g + 1) * 128], h_u[u][:],
                            start=False, stop=True,
                            skip_group_check=True)
                    nc.tensor.matmul(pdum[:], uq[u][:, 0:128], wq[u][:],
                                     start=True, stop=True,
                                     skip_group_check=True)
                    s_t = stp.tile([128, G4], F32, tag=f"s{u}",
                                   name=f"s{u}")
                    nc.scalar.activation(s_t[:, 0:256], ps[:, 0:256],
                                         AF.Sigmoid)
                    nc.scalar.activation(s_t[:, 256:512], ps[:, 256:512],
                                         AF.Sigmoid)
                    sg = s_t[:].rearrange("p (g r) -> p g r", g=4)
                    s_cc, s_i, s_f, s_o = (sg[:, g] for g in range(4))

                    q_t = stp.tile([128, B], F32, tag=f"q{u}", name=f"q{u}")
                    cf_t = stp.tile([128, B], F32, tag=f"cf{u}",
                                    name=f"cf{u}")
                    nc.vector.scalar_tensor_tensor(
                        q_t[:], s_cc, 0.5, s_i,
                        op0=OP.subtract, op1=OP.mult)
                    nc.vector.tensor_tensor(cf_t[:], s_f, c_u[u][:],
                                            op=OP.mult)
                    saved = None
                    if (u, j) in fix_map:
                        r = fix_map[(u, j)]
                        csave = stp.tile([128, B], F32, tag="csave")
                        hsave = stp.tile([128, B], BF16, tag="hsave")
                        nc.vector.tensor_copy(csave[:], c_u[u][:])
                        nc.vector.tensor_copy(hsave[:], h_u[u][:])
                        saved = (csave, hsave, r)
                    nc.vector.scalar_tensor_tensor(
                        c_u[u][:], q_t[:], 2.0, cf_t[:],
                        op0=OP.mult, op1=OP.add)
                    if saved:
                        csave, hsave, r = saved
                        nc.vector.copy_predicated(
                            c_u[u][:], mfix_sb[:, r * B:(r + 1) * B],
                            csave[:])
                    v_t = stp.tile([128, B], F32, tag=f"v{u}", name=f"v{u}")
                    nc.scalar.activation(v_t[:], c_u[u][:], AF.Sigmoid,
                                         scale=2.0)
                    nc.vector.scalar_tensor_tensor(
                        h_u[u][:], v_t[:], 0.5, s_o,
                        op0=OP.subtract, op1=OP.mult)
                    if saved:
                        csave, hsave, r = saved
                        nc.vector.copy_predicated(
                            h_u[u][:], mfix_sb[:, r * B:(r + 1) * B],
                            hsave[:])

                TRPRE = 4  # transpose lookahead (pairs): keeps the
                # in-order tensor queue from stalling on gather completions
                firstneed = {}
                for j in range(L):
                    for u in range(UPC):
                        b = blkmap(u, j)
                        if b not in firstneed:
                            firstneed[b] = j
                tr_at = {}
                for b, j in firstneed.items():
                    tr_at.setdefault(max(j - TRPRE, -1), []).append(b)
                psq = {u: [] for u in range(UPC)}
                for b in sorted(tr_at.get(-1, []),
                                key=lambda b: firstneed[b]):
                    emit_tr(b)
                for u in range(UPC):
                    psq[u].append(emit_wx(u, 0))
                for j in range(L):
                    for b in tr_at.get(j, []):
                        emit_tr(b)
                    for u in range(UPC):
                        if j + 1 < L:
                            psq[u].append(emit_wx(u, j + 1))
                        recurrent_step(u, j, psq[u][j])

                # ---- exchange final states (single masked AllReduce) ----
                hcat = sp.tile([H, 2 * B], F32, tag="hcat")
                for d2 in range(2):
                    dst = hcat[:, d2 * B:(d2 + 1) * B]
                    nc.vector.tensor_scalar(
                        dst, h_u[0][:], sel_sb[:, d2:d2 + 1], None,
                        op0=OP.mult)
                    for u in range(1, UPC):
                        nc.vector.scalar_tensor_tensor(
                            dst, h_u[u][:],
                            sel_sb[:, 2 * u + d2:2 * u + d2 + 1], dst,
                            op0=OP.mult, op1=OP.add)
                cc_in = dp.tile([H, 2 * B], F32, tag="cci")
                cc_out = dp.tile([H, 2 * B], F32, tag="cco")
                nc.sync.dma_start(cc_in[:, :], hcat[:])
                nc.gpsimd.collective_compute(
                    "AllReduce", OP.add,
                    replica_groups=[list(range(NCORES))],
                    ins=[cc_in.opt()], outs=[cc_out.opt()])
                hfull = sp.tile([H, 2 * B], F32, tag="hfull")
                nc.sync.dma_start(hfull[:], cc_out[:, :])

                # ---- BN2 fold + dense + softmax (all 128 examples) ----
                st2 = sp.tile([H, 12], F32, tag="st2")
                scr2 = sp.tile([H, B], F32, tag="scr2")
                hn = sp.tile([H, 2 * B], BF16, tag="hn")
                for d2 in range(2):
                    hd = hfull[:, d2 * B:(d2 + 1) * B]
                    nc.vector.tensor_reduce(st2[:, 0:1], hd,
                                            axis=AX.X, op=OP.add)
                    nc.scalar.activation(scr2[:], hd, AF.Square,
                                         accum_out=st2[:, 1:2])
                    m2 = st2[:, 8:9]
                    v2 = st2[:, 9:10]
                    a2 = st2[:, 10:11]
                    of2 = st2[:, 11:12]
                    nc.vector.tensor_scalar(m2, st2[:, 0:1], 1.0 / B, None,
                                            op0=OP.mult)
                    nc.vector.tensor_scalar(v2, st2[:, 1:2], 1.0 / B, None,
                                            op0=OP.mult)
                    nc.vector.tensor_tensor(a2, m2, m2, op=OP.mult)
                    nc.vector.tensor_tensor(v2, v2, a2, op=OP.subtract)
                    # h halved: eps/4 reproduces BN(h) with eps exactly
                    nc.vector.tensor_scalar(v2, v2, BN_EPS / 4.0, None,
                                            op0=OP.add)
                    nc.scalar.activation(v2, v2, AF.Sqrt)
                    nc.vector.reciprocal(v2, v2)
                    nc.vector.tensor_tensor(a2, g2_sb[:, d2:d2 + 1], v2,
                                            op=OP.mult)
                    nc.vector.tensor_tensor(of2, a2, m2, op=OP.mult)
                    nc.vector.tensor_tensor(of2, be2_sb[:, d2:d2 + 1], of2,
                                            op=OP.subtract)
                    nc.vector.tensor_scalar(hn[:, d2 * B:(d2 + 1) * B], hd,
                                            a2, of2, op0=OP.mult, op1=OP.add)

                ps_o = po.tile([B, ODIM], F32, space="PSUM")
                nc.tensor.matmul(ps_o[:], hn[:, 0:B], wdq[0][:],
                                 start=True, stop=False,
                                 skip_group_check=True)
                nc.tensor.matmul(ps_o[:], hn[:, B:2 * B], wdq[1][:],
                                 start=False, stop=True,
                                 skip_group_check=True)
                z = sp.tile([B, ODIM], F32, tag="z")
                ez = sp.tile([B, ODIM], F32, tag="ez")
                mx = sp.tile([B, 2], F32, tag="mx")
                nc.vector.tensor_tensor(z[:], ps_o[:], bd_sb[:], op=OP.add)
                nc.vector.tensor_reduce(mx[:, 0:1], z[:], axis=AX.X,
                                        op=OP.max)
                nc.vector.tensor_scalar(mx[:, 1:2], mx[:, 0:1], -1.0, None,
                                        op0=OP.mult)
                nc.scalar.activation(ez[:], z[:], AF.Exp, bias=mx[:, 1:2],
                                     accum_out=mx[:, 0:1])
                nc.vector.reciprocal(mx[:, 0:1], mx[:, 0:1])
                nc.vector.tensor_scalar(z[:], ez[:], mx[:, 0:1], None,
                                        op0=OP.mult)
                nc.sync.dma_start(out_d[:, :], z[:])

    nc.finalize()
    return nc


def _block_tokens(core):
    """block index -> token time t for this core (or None for dummy)."""
    toks = []
    for b in range(SEG):
        toks.append(SEG * core + b)
    for i in range(WARM):                       # fwd warmup, ascending t
        t = SEG * core - WARM + i
        toks.append(t if t >= 0 else None)
    for i in range(WARM):                       # bwd warmup, descending t
        t = SEG * (core + 1) + WARM - 1 - i
        toks.append(t if t < T else None)
    return toks


def _core_ids(inputs, core):
    ids = np.asarray(inputs["ids"]).astype(np.int64)  # [B, T]
    out = np.zeros((B, NBLK), np.int64)
    for b, t in enumerate(_block_tokens(core)):
        if t is not None:
            out[:, b] = ids[:, t]
    return out


def _fold_weights(inputs):
    ids = np.asarray(inputs["ids"]).astype(np.int64)
    emb = np.asarray(inputs["embed_table"], np.float64)
    counts = np.bincount(ids.ravel(), minlength=VOCAB).astype(np.float64)
    n = float(B * T)
    sum_x = counts @ emb
    sumsq_x = counts @ (emb * emb)
    m1 = sum_x / n
    v1 = sumsq_x / n - m1 * m1
    a1 = np.asarray(inputs["gamma1"], np.float64) / np.sqrt(v1 + BN_EPS)
    cvec = np.asarray(inputs["beta1"], np.float64) - a1 * m1

    folded = {}
    for d, (wk, uk, bk) in enumerate([("Wf", "Uf", "bf"), ("Wb", "Ub", "bb")]):
        W = np.asarray(inputs[wk], np.float64)
        U = np.asarray(inputs[uk], np.float64)
        b = np.asarray(inputs[bk], np.float64)
        Wp = W * a1[:, None]
        bp = b + cvec @ W
        Up = U * 2.0
        Wp[:, 256:384] *= 2.0
        Up[:, 256:384] *= 2.0
        bp[256:384] *= 2.0
        # permute gate blocks [i,f,cc,o] -> [cc,i,f,o] (split-sigmoid order)
        perm = [2, 0, 1, 3]
        Wp = np.concatenate([Wp[:, 128 * p:128 * (p + 1)] for p in perm],
                            axis=1)
        Up = np.concatenate([Up[:, 128 * p:128 * (p + 1)] for p in perm],
                            axis=1)
        bp = np.concatenate([bp[128 * p:128 * (p + 1)] for p in perm])
        folded[d] = (
            np.ascontiguousarray(
                Wp.astype(np.float32).astype(ml_dtypes.bfloat16)),
            np.ascontiguousarray(
                Up.astype(np.float32).astype(ml_dtypes.bfloat16)),
            np.ascontiguousarray(
                bp.astype(np.float32).astype(ml_dtypes.bfloat16)
                .reshape(4, 128)),
        )
    return folded


def kernel(**inputs):
    global LAST_RESULT

    core_ids = [_core_ids(inputs, c) for c in range(NCORES)]
    sched = set()
    for c in range(NCORES):
        for u in range(UPC):
            for j in range(L):
                if (core_ids[c][:, blkmap(u, j)] == 0).any():
                    sched.add((u, int(j)))
    mask_sched = sorted(sched)
    NFIX = len(mask_sched)

    nc = build_program(mask_sched)

    folded = _fold_weights(inputs)
    emb_bf = np.ascontiguousarray(
        np.asarray(inputs["embed_table"], np.float32)
        .astype(ml_dtypes.bfloat16))
    Wd = np.asarray(inputs["Wd"], np.float32)
    com = {
        "emb": emb_bf,
        "g2": np.ascontiguousarray(
            np.asarray(inputs["gamma2"], np.float32).reshape(2, H).T),
        "be2": np.ascontiguousarray(
            np.asarray(inputs["beta2"], np.float32).reshape(2, H).T),
        "Wd0": np.ascontiguousarray(Wd[0:H, :].astype(ml_dtypes.bfloat16)),
        "Wd1": np.ascontiguousarray(
            Wd[H:2 * H, :].astype(ml_dtypes.bfloat16)),
        "bd": np.ascontiguousarray(
            np.broadcast_to(np.asarray(inputs["bd"], np.float32),
                            (B, ODIM))),
    }

    in_maps = []
    for c in range(NCORES):
        m = dict(com)
        m["ids"] = np.ascontiguousarray(core_ids[c].astype(np.int32))
        sel = np.zeros((H, 2 * UPC), np.float32)
        if c == NCORES - 1:
            sel[:, 0] = 1.0        # fwd final lives on core 7 (slot 0)
        if c == 0:
            sel[:, 3] = 1.0        # bwd final lives on core 0 (slot 1)
        for u in range(UPC):
            W_b, U_b, Bp_b = folded[u]   # slot 0 = fwd dir, slot 1 = bwd
            m[f"W{u}"] = W_b
            m[f"U{u}"] = U_b
            m[f"Bp{u}"] = Bp_b
        m["sel"] = sel
        if NFIX:
            mf = np.zeros((NFIX, 128, B), np.uint8)
            for r, (u, j) in enumerate(mask_sched):
                inv = (core_ids[c][:, blkmap(u, j)] == 0).astype(np.uint8)
                mf[r, :, :] = inv[None, :]
            m["mfix"] = mf.reshape(NFIX * 128, B)
        in_maps.append(m)

    res = run_bass_kernel_spmd(nc, in_maps, list(range(NCORES)),
                               trace=TRACE, tmpdir=TRACE_DIR)
    LAST_RESULT = {"exec_time_ns": res.exec_time_ns}
    return np.asarray(res.results[0]["out"]).astype(np.float32)
